# revision 19
# baseline (speedup 1.0000x reference)
"""BayesianLinear (reparameterized sampling + linear) on 8 TRN2 NeuronCores.

Math:  w = weight_mu + weight_eps * exp(0.5*weight_log_var)   [OUT_F, IN_F]
       b = bias_mu + bias_eps * exp(0.5*bias_log_var)         [OUT_F]
       out = x @ w.T + b                                      [BATCH, OUT_F]

Sharding: 2-way over BATCH x 4-way over OUT_F.  Each core computes its
[O_core, B_core] output tile TRANSPOSED (o on partitions); the host
transposes while gathering (free).

Kernel structure (v3, operand-swapped):
  - The sampled weight tile w[128k, 128o] is the STATIONARY matmul
    operand; x[128k, 512b] is the MOVING operand.  Four consecutive
    matmuls (4 batch blocks) share one stationary tile, so LDWEIGHTS
    amortizes 4x (measured: unshared costs ~46 ns/MM, pair-shared
    ~6 ns/MM).
  - Every weight chunk feeds 64 matmuls (all batch), so the weight
    stream never paces the PE (arrival margin ~3x) — no special
    startup phase.
  - PSUM can't hold the 64 accumulation chains, so 4-k-tile windows
    spill into SBUF fp16 accumulators (rel-err measured 2.4e-3 on the
    spec inputs, vs 3.3e-3 for the all-PSUM bf16 baseline).  Each
    group accumulates in one 4-bank [128, 2048] PSUM tile and spills
    with a single wide DVE op; bias folds into the window-0 spill.
  - x DMAs are one k-tile x full B_core: 8 KB/partition contiguous.
  - Output DMAs issue on the ACT HWDGE queue so they never head-block
    the SP input-stream queue; the last window's outputs stream out
    per-chain, overlapped with its own matmuls.
"""

import numpy as np
import ml_dtypes

BATCH = 8192
IN_F = 4096
OUT_F = 4096
B_SHARDS = 2
O_SHARDS = 4
N_CORES = B_SHARDS * O_SHARDS

B_CORE = BATCH // B_SHARDS   # 4096
O_CORE = OUT_F // O_SHARDS   # 1024

BF16 = ml_dtypes.bfloat16

_PROGRAM_CACHE = {}


def build_program(B_core=B_CORE, O_core=O_CORE, K=IN_F):
    """Build + compile the per-core Bass/Tile program (same NEFF on all cores).

    DRAM parameters (per core):
      xt   [K, B_core]  bf16   x shard, transposed (K-major)
      wmu  [K, O_core]  bf16   weight_mu shard, transposed
      wlv  [K, O_core]  bf16   weight_log_var shard, transposed
      weps [K, O_core]  bf16   weight_eps shard, transposed
      bstack [128, 3*OT] f32   bias shard, o-major per partition:
                               cols [0,OT)=log_var [OT,2*OT)=eps
                               [2*OT,3*OT)=mu  (one contiguous DMA —
                               a [O_core,1] layout DMAs as 1024
                               4-byte descriptors and wrecks the
                               early stream)
      out  [O_core, B_core] f32      TRANSPOSED output tile
    """
    import concourse.mybir as mybir
    import concourse.tile as tile
    from concourse import bacc

    assert K % 128 == 0 and B_core % 512 == 0 and O_core % 128 == 0
    KT = K // 128          # contraction k-tiles (32)
    OT = O_core // 128     # o sub-tiles (8)
    NBB = B_core // 512    # batch blocks (8)
    assert NBB % 4 == 0

    f32 = mybir.dt.float32
    f16 = mybir.dt.float16
    bf16 = mybir.dt.bfloat16
    Exp = mybir.ActivationFunctionType.Exp
    mult = mybir.AluOpType.mult
    add = mybir.AluOpType.add

    nc = bacc.Bacc("TRN2", target_bir_lowering=False, debug=False)

    xt = nc.declare_dram_parameter("xt", [K, B_core], bf16, isOutput=False)
    wmu = nc.declare_dram_parameter("wmu", [K, O_core], bf16, isOutput=False)
    wlv = nc.declare_dram_parameter("wlv", [K, O_core], bf16, isOutput=False)
    weps = nc.declare_dram_parameter("weps", [K, O_core], bf16, isOutput=False)
    bstack = nc.declare_dram_parameter("bstack", [128, 3 * OT], f32,
                                       isOutput=False)
    out = nc.declare_dram_parameter("out", [O_core, B_core], f32, isOutput=True)

    xt_r = xt.ap().rearrange("(kt p) b -> p kt b", p=128)
    out_r = out.ap().rearrange("(ot p) b -> p ot b", p=128)
    wmu_r = wmu.ap().rearrange("(c p) o -> p c o", p=128)
    wlv_r = wlv.ap().rearrange("(c p) o -> p c o", p=128)
    weps_r = weps.ap().rearrange("(c p) o -> p c o", p=128)

    # Weight-stage chunks (k-tiles): small leading chunks start the first
    # matmuls early; 2-tile chunks keep the stage buffers small.
    WSIZES = [1, 1, 2] + [2] * ((KT - 4) // 2)
    assert sum(WSIZES) == KT
    WSTARTS = [sum(WSIZES[:i]) for i in range(len(WSIZES))]
    K2C = []
    for ci, (s, st) in enumerate(zip(WSIZES, WSTARTS)):
        K2C += [(ci, k - st) for k in range(st, st + s)]

    # Spill windows (k-tiles).  7 fp16 round-trips on the accumulators.
    WINDOWS = [(k, k + 4) for k in range(0, KT, 4)]

    with tile.TileContext(nc) as tc:
        with (
            tc.tile_pool(name="wres", bufs=1) as wres_pool,
            tc.tile_pool(name="wstage", bufs=2) as wstage_pool,
            tc.tile_pool(name="xblk", bufs=8) as xpool,
            tc.tile_pool(name="osb", bufs=6) as opool,
            tc.tile_pool(name="acc", bufs=1) as acc_pool,
            tc.tile_pool(name="bias", bufs=1) as bias_pool,
            tc.tile_pool(name="psum", bufs=2, space="PSUM") as ppool,
        ):
            # ---- bias: b = bmu + beps * exp(0.5*blv), o-major [128, OT].
            # Emitted lazily so its DMAs stay behind the first chunks; only
            # needed by the last window, ~380 us in.
            def emit_bias():
                bstage = bias_pool.tile([128, 3 * OT], f32, tag="bstage",
                                        name="bstage")
                b_lv = bstage[:, 0:OT]
                b_eps = bstage[:, OT:2 * OT]
                b_mu = bstage[:, 2 * OT:3 * OT]
                nc.sync.dma_start(out=bstage[:], in_=bstack.ap())
                nc.scalar.activation(b_lv, b_lv, Exp, scale=0.5)
                nc.vector.tensor_tensor(out=b_lv, in0=b_lv, in1=b_eps, op=mult)
                nc.vector.tensor_tensor(out=b_lv, in0=b_lv, in1=b_mu, op=add)
                return bstage

            def load_w_chunk(ci):
                size, st = WSIZES[ci], WSTARTS[ci]
                ksl = slice(st, st + size)
                # DMA order = critical-path order: exp needs lv first,
                # then mult needs eps, add needs mu last.
                slv = wstage_pool.tile([128, size, O_core], bf16, tag="slv",
                                       name=f"slv_{ci}")
                seps = wstage_pool.tile([128, size, O_core], bf16, tag="seps",
                                        name=f"seps_{ci}")
                smu = wstage_pool.tile([128, size, O_core], bf16, tag="smu",
                                       name=f"smu_{ci}")
                # Weight staging runs on the ACT HWDGE queue (where the
                # sampling exp already lives) so it streams in parallel
                # with the x tiles on the SP queue.
                nc.scalar.dma_start(out=slv[:], in_=wlv_r[:, ksl, :])
                nc.scalar.dma_start(out=seps[:], in_=weps_r[:, ksl, :])
                nc.scalar.dma_start(out=smu[:], in_=wmu_r[:, ksl, :])
                # A sampled chunk is consumed by one window's matmuls and
                # then dead, so a short ring suffices (~3 windows deep).
                small = size < max(WSIZES)
                w_c = wres_pool.tile(
                    [128, size, O_core], bf16,
                    tag=f"wres_s{ci}" if small else "wres",
                    bufs=(1 if small else 6),
                    name=f"wres_{ci}")
                nc.scalar.activation(w_c[:], slv[:], Exp, scale=0.5)
                nc.vector.tensor_tensor(out=w_c[:], in0=w_c[:], in1=seps[:],
                                        op=mult)
                nc.vector.tensor_tensor(out=w_c[:], in0=w_c[:], in1=smu[:],
                                        op=add)
                return w_c

            wchunks = []

            def wres_slice(k, ot):
                ci, off = K2C[k]
                return wchunks[ci][:, off, ot * 128:(ot + 1) * 128]

            xtiles = {}

            def need_x(kt):
                if kt not in xtiles:
                    t = xpool.tile([128, B_core], bf16, tag="xblk",
                                   name=f"xblk_{kt}")
                    nc.sync.dma_start(out=t[:], in_=xt_r[:, kt, :])
                    xtiles[kt] = t
                return xtiles[kt]

            # fp16 spill accumulators: one [o128, b2048] chain per (ot, bq).
            # Bias is folded in at the window-0 spill (it only depends on
            # the partition o, so a per-partition tensor_scalar add).
            accs = {}
            for ot in range(OT):
                for bq in range(NBB // 4):
                    accs[(ot, bq)] = acc_pool.tile(
                        [128, 2048], f16, tag="acc", bufs=OT * (NBB // 4),
                        name=f"acc_{ot}_{bq}")

            state = {"next_w": 0}

            def emit_loads(k1):
                # Loads (DMA issue + sampling ops) for weight chunks up to
                # k-tile k1, each followed by the x k-tiles it covers.
                while (state["next_w"] < len(WSIZES)
                       and WSTARTS[state["next_w"]] < k1):
                    ci = state["next_w"]
                    wchunks.append(load_w_chunk(ci))
                    for kt in range(WSTARTS[ci], WSTARTS[ci] + WSIZES[ci]):
                        need_x(kt)
                    state["next_w"] += 1

            n_win = len(WINDOWS)
            emit_loads(WSTARTS[1])        # chunk 0 + x k-tile 0
            bias_sb = emit_bias()         # tiny DMAs, needed by window 0
            emit_loads(WINDOWS[0][1])     # rest of window 0
            for wi, (k0, k1) in enumerate(WINDOWS):
                first, last = (wi == 0), (wi == n_win - 1)
                for ot in range(OT):
                    for bq in range(NBB // 4):
                        # Hoist the NEXT window's loads into this window's
                        # instruction stream: the sampling ops land in the
                        # DVE/ACT FIFOs mid-window, so the next window's
                        # first matmuls never wait on them, and its x tiles
                        # (each group sweeps the full k-window in ~3.5 us)
                        # are on SBUF at the boundary.
                        if ot == 1 and bq == 0 and wi + 1 < n_win:
                            emit_loads(WINDOWS[wi + 1][1])
                        bbs = range(4 * bq, 4 * bq + 4)
                        # One 4-bank PSUM tile per group: 4 interleaved
                        # accumulation chains, spilled with a single wide
                        # DVE op (the ~0.5 us/op PSUM-read overhead made
                        # per-bank spills the bottleneck).
                        ps = ppool.tile([128, 2048], f32, tag="ps",
                                        name=f"ps_{wi}_{ot}_{bq}")
                        for k in range(k0, k1):
                            for i, bb in enumerate(bbs):
                                nc.tensor.matmul(
                                    ps[:, i * 512:(i + 1) * 512],
                                    wres_slice(k, ot),
                                    xtiles[k][:, bb * 512:(bb + 1) * 512],
                                    start=(k == k0),
                                    stop=(k == k1 - 1),
                                )
                        a = accs[(ot, bq)]
                        if first:
                            nc.vector.tensor_scalar_add(
                                out=a[:], in0=ps[:],
                                scalar1=bias_sb[:, ot:ot + 1])
                        elif not last:
                            nc.vector.tensor_tensor(
                                out=a[:], in0=ps[:], in1=a[:], op=add)
                        else:
                            # Final window: per-bank finals so the out
                            # stream starts earlier and the very last
                            # transfer is 256 KB, not 1 MB.
                            for i, bb in enumerate(bbs):
                                osb = opool.tile([128, 512], f32, tag="osb",
                                                 name=f"osb_{ot}_{bb}")
                                nc.vector.tensor_tensor(
                                    out=osb[:],
                                    in0=ps[:, i * 512:(i + 1) * 512],
                                    in1=a[:, i * 512:(i + 1) * 512], op=add)
                                # ACT HWDGE queue: output DMAs never
                                # head-block the SP input-stream queue.
                                nc.scalar.dma_start(
                                    out=out_r[:, ot, bb * 512:(bb + 1) * 512],
                                    in_=osb[:])

    nc.compile()
    return nc


def _get_program():
    key = (B_CORE, O_CORE, IN_F)
    if key not in _PROGRAM_CACHE:
        _PROGRAM_CACHE[key] = build_program(*key)
    return _PROGRAM_CACHE[key]


def make_in_maps(x, weight_mu, weight_log_var, bias_mu, bias_log_var,
                 weight_eps, bias_eps):
    """Shard + transpose + cast the full inputs into 8 per-core input maps."""
    x = np.asarray(x, dtype=np.float32)
    weight_mu = np.asarray(weight_mu, dtype=np.float32)
    weight_log_var = np.asarray(weight_log_var, dtype=np.float32)
    weight_eps = np.asarray(weight_eps, dtype=np.float32)
    bias_mu = np.asarray(bias_mu, dtype=np.float32).reshape(-1)
    bias_log_var = np.asarray(bias_log_var, dtype=np.float32).reshape(-1)
    bias_eps = np.asarray(bias_eps, dtype=np.float32).reshape(-1)

    xt = np.ascontiguousarray(x.astype(BF16).T)              # [IN_F, BATCH]
    wmu_t = np.ascontiguousarray(weight_mu.astype(BF16).T)   # [IN_F, OUT_F]
    wlv_t = np.ascontiguousarray(weight_log_var.astype(BF16).T)
    weps_t = np.ascontiguousarray(weight_eps.astype(BF16).T)

    OT = O_CORE // 128
    in_maps = []
    for c in range(N_CORES):
        bi, oi = divmod(c, O_SHARDS)
        bs = slice(bi * B_CORE, (bi + 1) * B_CORE)
        os_ = slice(oi * O_CORE, (oi + 1) * O_CORE)
        # o-major per partition: bstack[p, j*OT + ot] = tensor_j[ot*128 + p]
        bstack = np.concatenate(
            [bias_log_var[os_].reshape(OT, 128).T,
             bias_eps[os_].reshape(OT, 128).T,
             bias_mu[os_].reshape(OT, 128).T], axis=1)
        in_maps.append({
            "xt": np.ascontiguousarray(xt[:, bs]),
            "wmu": np.ascontiguousarray(wmu_t[:, os_]),
            "wlv": np.ascontiguousarray(wlv_t[:, os_]),
            "weps": np.ascontiguousarray(weps_t[:, os_]),
            "bstack": np.ascontiguousarray(bstack, dtype=np.float32),
        })
    return in_maps


def gather_output(results):
    out = np.empty((BATCH, OUT_F), dtype=np.float32)
    for c in range(N_CORES):
        bi, oi = divmod(c, O_SHARDS)
        out[bi * B_CORE:(bi + 1) * B_CORE, oi * O_CORE:(oi + 1) * O_CORE] = \
            results[c]["out"].T
    return out


def run_on_hw(in_maps, trace=False):
    from concourse.bass_utils import run_bass_kernel_spmd
    nc = _get_program()
    return run_bass_kernel_spmd(nc, in_maps, list(range(N_CORES)), trace=trace)


_RUNNER = None


def _get_runner():
    """Build (once per process) a cached jit callable: in_maps -> results.

    Mirrors bass2jax.run_bass_via_pjrt's multi-core branch, but keeps the
    jitted executable alive so repeated kernel() calls skip recompilation.
    """
    global _RUNNER
    if _RUNNER is not None:
        return _RUNNER
    import jax
    from jax.sharding import Mesh, PartitionSpec
    try:
        from jax.experimental.shard_map import shard_map
    except ImportError:  # newer jax
        from jax import shard_map
    import concourse.mybir as mybir
    from concourse import bass2jax

    nc = _get_program()
    bass2jax.install_neuronx_cc_hook()
    assert nc.dbg_addr is None
    partition_name = (nc.partition_id_tensor.name
                      if nc.partition_id_tensor else None)

    in_names, out_names, out_shapes, out_dtypes = [], [], [], []
    for alloc in nc.m.functions[0].allocations:
        if not isinstance(alloc, mybir.MemoryLocationSet):
            continue
        name = alloc.memorylocations[0].name
        if alloc.kind == "ExternalInput":
            if name != partition_name:
                in_names.append(name)
        elif alloc.kind == "ExternalOutput":
            out_names.append(name)
            out_shapes.append(tuple(alloc.tensor_shape))
            out_dtypes.append(mybir.dt.np(alloc.dtype))
    out_avals = [jax.core.ShapedArray(s, d)
                 for s, d in zip(out_shapes, out_dtypes)]
    n_params = len(in_names)
    all_names = list(in_names + out_names)
    if partition_name is not None:
        all_names.append(partition_name)
    all_names = tuple(all_names)

    def _body(*args):
        operands = list(args)
        if partition_name is not None:
            operands.append(bass2jax.partition_id_tensor())
        outs = bass2jax._bass_exec_p.bind(
            *operands,
            out_avals=tuple(out_avals),
            in_names=all_names,
            out_names=tuple(out_names),
            lowering_input_output_aliases=(),
            sim_require_finite=True,
            sim_require_nnan=True,
            nc=nc,
        )
        return tuple(outs)

    devices = jax.devices()[:N_CORES]
    assert len(devices) == N_CORES
    mesh = Mesh(np.asarray(devices), ("core",))
    donate = tuple(range(n_params, n_params + len(out_names)))
    sharded = jax.jit(
        shard_map(
            _body, mesh=mesh,
            in_specs=(PartitionSpec("core"),) * (n_params + len(out_names)),
            out_specs=(PartitionSpec("core"),) * len(out_names),
            check_rep=False),
        donate_argnums=donate, keep_unused=True)

    def run(in_maps):
        per_core = [[np.asarray(m[name]) for name in in_names]
                    for m in in_maps]
        concat_in = [
            np.concatenate([per_core[c][i] for c in range(N_CORES)], axis=0)
            for i in range(n_params)
        ]
        zero_outs = [np.zeros((N_CORES * s[0],) + s[1:], d)
                     for s, d in zip(out_shapes, out_dtypes)]
        outs = sharded(*concat_in, *zero_outs)
        results = []
        for c in range(N_CORES):
            m = {}
            for i, name in enumerate(out_names):
                s0 = out_shapes[i][0]
                m[name] = np.asarray(outs[i][c * s0:(c + 1) * s0])
            results.append(m)
        return results

    _RUNNER = run
    return run


def kernel(x, weight_mu, weight_log_var, bias_mu, bias_log_var,
           weight_eps, bias_eps):
    in_maps = make_in_maps(x, weight_mu, weight_log_var, bias_mu,
                           bias_log_var, weight_eps, bias_eps)
    results = _get_runner()(in_maps)
    return gather_output(results)


# revision 23
# speedup vs baseline: 1.1765x; 1.1765x over previous
"""BayesianLinear (reparameterized sampling + linear) on 8 TRN2 NeuronCores.

Math:  w = weight_mu + weight_eps * exp(0.5*weight_log_var)   [OUT_F, IN_F]
       b = bias_mu + bias_eps * exp(0.5*bias_log_var)         [OUT_F]
       out = x @ w.T + b                                      [BATCH, OUT_F]

Sharding: 2-way over BATCH x 4-way over OUT_F.  Each core computes its
[O_core, B_core] output tile TRANSPOSED (o on partitions); the host
transposes while gathering (free).

Kernel structure (v3, operand-swapped):
  - The sampled weight tile w[128k, 128o] is the STATIONARY matmul
    operand; x[128k, 512b] is the MOVING operand.  Four consecutive
    matmuls (4 batch blocks) share one stationary tile, so LDWEIGHTS
    amortizes 4x (measured: unshared costs ~46 ns/MM, pair-shared
    ~6 ns/MM).
  - Every weight chunk feeds 64 matmuls (all batch), so the weight
    stream never paces the PE (arrival margin ~3x) — no special
    startup phase.
  - PSUM can't hold the 64 accumulation chains, so 4-k-tile windows
    spill into SBUF fp16 accumulators (rel-err measured 2.4e-3 on the
    spec inputs, vs 3.3e-3 for the all-PSUM bf16 baseline).  Each
    group accumulates in one 4-bank [128, 2048] PSUM tile and spills
    with a single wide DVE op; bias folds into the window-0 spill.
  - x DMAs are one k-tile x full B_core: 8 KB/partition contiguous.
  - Output DMAs issue on the ACT HWDGE queue so they never head-block
    the SP input-stream queue; the last window's outputs stream out
    per-chain, overlapped with its own matmuls.
"""

import numpy as np
import ml_dtypes

BATCH = 8192
IN_F = 4096
OUT_F = 4096
B_SHARDS = 2
O_SHARDS = 4
N_CORES = B_SHARDS * O_SHARDS

B_CORE = BATCH // B_SHARDS   # 4096
O_CORE = OUT_F // O_SHARDS   # 1024

BF16 = ml_dtypes.bfloat16

_PROGRAM_CACHE = {}


def build_program(B_core=B_CORE, O_core=O_CORE, K=IN_F):
    """Build + compile the per-core Bass/Tile program (same NEFF on all cores).

    DRAM parameters (per core):
      xt   [K, B_core]  bf16   x shard, transposed (K-major)
      wmu  [K, O_core]  bf16   weight_mu shard, transposed
      wlv  [K, O_core]  bf16   weight_log_var shard, transposed
      weps [K, O_core]  bf16   weight_eps shard, transposed
      bstack [128, 3*OT] f32   bias shard, o-major per partition:
                               cols [0,OT)=log_var [OT,2*OT)=eps
                               [2*OT,3*OT)=mu  (one contiguous DMA —
                               a [O_core,1] layout DMAs as 1024
                               4-byte descriptors and wrecks the
                               early stream)
      out  [O_core, B_core] f32      TRANSPOSED output tile
    """
    import concourse.mybir as mybir
    import concourse.tile as tile
    from concourse import bacc

    assert K % 128 == 0 and B_core % 512 == 0 and O_core % 128 == 0
    KT = K // 128          # contraction k-tiles (32)
    OT = O_core // 128     # o sub-tiles (8)
    NBB = B_core // 512    # batch blocks (8)
    assert NBB % 4 == 0

    f32 = mybir.dt.float32
    f16 = mybir.dt.float16
    bf16 = mybir.dt.bfloat16
    Exp = mybir.ActivationFunctionType.Exp
    mult = mybir.AluOpType.mult
    add = mybir.AluOpType.add

    nc = bacc.Bacc("TRN2", target_bir_lowering=False, debug=False)

    xt = nc.declare_dram_parameter("xt", [K, B_core], bf16, isOutput=False)
    wmu = nc.declare_dram_parameter("wmu", [K, O_core], bf16, isOutput=False)
    wlv = nc.declare_dram_parameter("wlv", [K, O_core], bf16, isOutput=False)
    weps = nc.declare_dram_parameter("weps", [K, O_core], bf16, isOutput=False)
    bstack = nc.declare_dram_parameter("bstack", [128, 3 * OT], f32,
                                       isOutput=False)
    out = nc.declare_dram_parameter("out", [O_core, B_core], f32, isOutput=True)

    xt_r = xt.ap().rearrange("(kt p) b -> p kt b", p=128)
    out_r = out.ap().rearrange("(ot p) b -> p ot b", p=128)
    wmu_r = wmu.ap().rearrange("(c p) o -> p c o", p=128)
    wlv_r = wlv.ap().rearrange("(c p) o -> p c o", p=128)
    weps_r = weps.ap().rearrange("(c p) o -> p c o", p=128)

    # Weight-stage chunks (k-tiles): small leading chunks start the first
    # matmuls early; 2-tile chunks keep the stage buffers small.
    WSIZES = [1, 1, 2] + [2] * ((KT - 4) // 2)
    assert sum(WSIZES) == KT
    WSTARTS = [sum(WSIZES[:i]) for i in range(len(WSIZES))]
    K2C = []
    for ci, (s, st) in enumerate(zip(WSIZES, WSTARTS)):
        K2C += [(ci, k - st) for k in range(st, st + s)]

    # Spill windows (k-tiles).  7 fp16 round-trips on the accumulators.
    WINDOWS = [(k, k + 4) for k in range(0, KT, 4)]

    with tile.TileContext(nc) as tc:
        with (
            tc.tile_pool(name="wres", bufs=1) as wres_pool,
            tc.tile_pool(name="wstage", bufs=2) as wstage_pool,
            tc.tile_pool(name="xblk", bufs=16) as xpool,
            tc.tile_pool(name="osb", bufs=6) as opool,
            tc.tile_pool(name="acc", bufs=1) as acc_pool,
            tc.tile_pool(name="bias", bufs=1) as bias_pool,
            tc.tile_pool(name="psum", bufs=2, space="PSUM") as ppool,
        ):
            # ---- bias: b = bmu + beps * exp(0.5*blv), o-major [128, OT].
            # Emitted lazily so its DMAs stay behind the first chunks; only
            # needed by the last window, ~380 us in.
            def emit_bias():
                bstage = bias_pool.tile([128, 3 * OT], f32, tag="bstage",
                                        name="bstage")
                b_lv = bstage[:, 0:OT]
                b_eps = bstage[:, OT:2 * OT]
                b_mu = bstage[:, 2 * OT:3 * OT]
                nc.sync.dma_start(out=bstage[:], in_=bstack.ap())
                nc.scalar.activation(b_lv, b_lv, Exp, scale=0.5)
                nc.vector.tensor_tensor(out=b_lv, in0=b_lv, in1=b_eps, op=mult)
                nc.vector.tensor_tensor(out=b_lv, in0=b_lv, in1=b_mu, op=add)
                return bstage

            def load_w_chunk(ci):
                size, st = WSIZES[ci], WSTARTS[ci]
                ksl = slice(st, st + size)
                # DMA order = critical-path order: exp needs lv first,
                # then mult needs eps, add needs mu last.
                slv = wstage_pool.tile([128, size, O_core], bf16, tag="slv",
                                       name=f"slv_{ci}")
                seps = wstage_pool.tile([128, size, O_core], bf16, tag="seps",
                                        name=f"seps_{ci}")
                smu = wstage_pool.tile([128, size, O_core], bf16, tag="smu",
                                       name=f"smu_{ci}")
                # Weight staging runs on the ACT HWDGE queue (where the
                # sampling exp already lives) so it streams in parallel
                # with the x tiles on the SP queue.
                nc.scalar.dma_start(out=slv[:], in_=wlv_r[:, ksl, :])
                nc.scalar.dma_start(out=seps[:], in_=weps_r[:, ksl, :])
                nc.scalar.dma_start(out=smu[:], in_=wmu_r[:, ksl, :])
                # A sampled chunk is consumed by one window's matmuls and
                # then dead, so a short ring suffices (~3 windows deep).
                small = size < max(WSIZES)
                w_c = wres_pool.tile(
                    [128, size, O_core], bf16,
                    tag=f"wres_s{ci}" if small else "wres",
                    bufs=(1 if small else 6),
                    name=f"wres_{ci}")
                nc.scalar.activation(w_c[:], slv[:], Exp, scale=0.5)
                nc.vector.tensor_tensor(out=w_c[:], in0=w_c[:], in1=seps[:],
                                        op=mult)
                nc.vector.tensor_tensor(out=w_c[:], in0=w_c[:], in1=smu[:],
                                        op=add)
                return w_c

            wchunks = []

            def wres_slice(k, ot):
                ci, off = K2C[k]
                return wchunks[ci][:, off, ot * 128:(ot + 1) * 128]

            # x in half-width tiles: group (ot, bq) reads only half bq, so
            # a window's first groups unlock after ~5 MB instead of 7 MB.
            xtiles = {}

            def need_x(kt, h):
                if (kt, h) not in xtiles:
                    t = xpool.tile([128, B_core // 2], bf16, tag="xblk",
                                   name=f"xblk_{kt}_{h}")
                    nc.sync.dma_start(
                        out=t[:],
                        in_=xt_r[:, kt, h * (B_core // 2):
                                 (h + 1) * (B_core // 2)])
                    xtiles[(kt, h)] = t
                return xtiles[(kt, h)]

            # fp16 spill accumulators: one [o128, b2048] chain per (ot, bq).
            # Bias is folded in at the window-0 spill (it only depends on
            # the partition o, so a per-partition tensor_scalar add).
            accs = {}
            for ot in range(OT):
                for bq in range(NBB // 4):
                    accs[(ot, bq)] = acc_pool.tile(
                        [128, 2048], f16, tag="acc", bufs=OT * (NBB // 4),
                        name=f"acc_{ot}_{bq}")

            state = {"next_w": 0}

            def emit_loads(k1):
                # Loads (DMA issue + sampling ops) for weight chunks up to
                # k-tile k1.  Order = consumption order: each chunk, then
                # the bq=0 x halves it covers; the bq=1 halves trail the
                # whole batch (their groups run second).
                kt_lo = (WSTARTS[state["next_w"]]
                         if state["next_w"] < len(WSIZES) else k1)
                while (state["next_w"] < len(WSIZES)
                       and WSTARTS[state["next_w"]] < k1):
                    ci = state["next_w"]
                    wchunks.append(load_w_chunk(ci))
                    for kt in range(WSTARTS[ci], WSTARTS[ci] + WSIZES[ci]):
                        need_x(kt, 0)
                    state["next_w"] += 1
                for kt in range(kt_lo, k1):
                    need_x(kt, 1)

            n_win = len(WINDOWS)
            emit_loads(WINDOWS[0][1])     # window-0 weights + x halves
            bias_sb = emit_bias()         # one tiny DMA, needed by window 0
            for wi, (k0, k1) in enumerate(WINDOWS):
                first, last = (wi == 0), (wi == n_win - 1)
                for g, (bq, ot) in enumerate(
                        (bq, ot) for bq in range(NBB // 4)
                        for ot in range(OT)):
                        # Hoist the NEXT window's loads into this window's
                        # instruction stream: the sampling ops land in the
                        # DVE/ACT FIFOs mid-window, so the next window's
                        # first matmuls never wait on them, and its x tiles
                        # (each group sweeps the full k-window in ~3.5 us)
                        # are on SBUF at the boundary.
                        if g == 2 and wi + 1 < n_win:
                            emit_loads(WINDOWS[wi + 1][1])
                        bbs = range(4 * bq, 4 * bq + 4)
                        # One 4-bank PSUM tile per group: 4 interleaved
                        # accumulation chains, spilled with a single wide
                        # DVE op (the ~0.5 us/op PSUM-read overhead made
                        # per-bank spills the bottleneck).
                        ps = ppool.tile([128, 2048], f32, tag="ps",
                                        name=f"ps_{wi}_{ot}_{bq}")
                        for k in range(k0, k1):
                            for i, bb in enumerate(bbs):
                                nc.tensor.matmul(
                                    ps[:, i * 512:(i + 1) * 512],
                                    wres_slice(k, ot),
                                    xtiles[(k, bq)][:, i * 512:(i + 1) * 512],
                                    start=(k == k0),
                                    stop=(k == k1 - 1),
                                )
                        a = accs[(ot, bq)]
                        if first:
                            nc.vector.tensor_scalar_add(
                                out=a[:], in0=ps[:],
                                scalar1=bias_sb[:, ot:ot + 1])
                        elif not last:
                            nc.vector.tensor_tensor(
                                out=a[:], in0=ps[:], in1=a[:], op=add)
                        else:
                            # Final window: per-bank finals so the out
                            # stream starts earlier and the very last
                            # transfer is 256 KB, not 1 MB.
                            for i, bb in enumerate(bbs):
                                osb = opool.tile([128, 512], f32, tag="osb",
                                                 name=f"osb_{ot}_{bb}")
                                nc.vector.tensor_tensor(
                                    out=osb[:],
                                    in0=ps[:, i * 512:(i + 1) * 512],
                                    in1=a[:, i * 512:(i + 1) * 512], op=add)
                                # ACT HWDGE queue: output DMAs never
                                # head-block the SP input-stream queue.
                                nc.scalar.dma_start(
                                    out=out_r[:, ot, bb * 512:(bb + 1) * 512],
                                    in_=osb[:])

    nc.compile()
    return nc


def _get_program():
    key = (B_CORE, O_CORE, IN_F)
    if key not in _PROGRAM_CACHE:
        _PROGRAM_CACHE[key] = build_program(*key)
    return _PROGRAM_CACHE[key]


def make_in_maps(x, weight_mu, weight_log_var, bias_mu, bias_log_var,
                 weight_eps, bias_eps):
    """Shard + transpose + cast the full inputs into 8 per-core input maps."""
    x = np.asarray(x, dtype=np.float32)
    weight_mu = np.asarray(weight_mu, dtype=np.float32)
    weight_log_var = np.asarray(weight_log_var, dtype=np.float32)
    weight_eps = np.asarray(weight_eps, dtype=np.float32)
    bias_mu = np.asarray(bias_mu, dtype=np.float32).reshape(-1)
    bias_log_var = np.asarray(bias_log_var, dtype=np.float32).reshape(-1)
    bias_eps = np.asarray(bias_eps, dtype=np.float32).reshape(-1)

    xt = np.ascontiguousarray(x.astype(BF16).T)              # [IN_F, BATCH]
    wmu_t = np.ascontiguousarray(weight_mu.astype(BF16).T)   # [IN_F, OUT_F]
    wlv_t = np.ascontiguousarray(weight_log_var.astype(BF16).T)
    weps_t = np.ascontiguousarray(weight_eps.astype(BF16).T)

    OT = O_CORE // 128
    in_maps = []
    for c in range(N_CORES):
        bi, oi = divmod(c, O_SHARDS)
        bs = slice(bi * B_CORE, (bi + 1) * B_CORE)
        os_ = slice(oi * O_CORE, (oi + 1) * O_CORE)
        # o-major per partition: bstack[p, j*OT + ot] = tensor_j[ot*128 + p]
        bstack = np.concatenate(
            [bias_log_var[os_].reshape(OT, 128).T,
             bias_eps[os_].reshape(OT, 128).T,
             bias_mu[os_].reshape(OT, 128).T], axis=1)
        in_maps.append({
            "xt": np.ascontiguousarray(xt[:, bs]),
            "wmu": np.ascontiguousarray(wmu_t[:, os_]),
            "wlv": np.ascontiguousarray(wlv_t[:, os_]),
            "weps": np.ascontiguousarray(weps_t[:, os_]),
            "bstack": np.ascontiguousarray(bstack, dtype=np.float32),
        })
    return in_maps


def gather_output(results):
    out = np.empty((BATCH, OUT_F), dtype=np.float32)
    for c in range(N_CORES):
        bi, oi = divmod(c, O_SHARDS)
        out[bi * B_CORE:(bi + 1) * B_CORE, oi * O_CORE:(oi + 1) * O_CORE] = \
            results[c]["out"].T
    return out


def run_on_hw(in_maps, trace=False):
    from concourse.bass_utils import run_bass_kernel_spmd
    nc = _get_program()
    return run_bass_kernel_spmd(nc, in_maps, list(range(N_CORES)), trace=trace)


_RUNNER = None


def _get_runner():
    """Build (once per process) a cached jit callable: in_maps -> results.

    Mirrors bass2jax.run_bass_via_pjrt's multi-core branch, but keeps the
    jitted executable alive so repeated kernel() calls skip recompilation.
    """
    global _RUNNER
    if _RUNNER is not None:
        return _RUNNER
    import jax
    from jax.sharding import Mesh, PartitionSpec
    try:
        from jax.experimental.shard_map import shard_map
    except ImportError:  # newer jax
        from jax import shard_map
    import concourse.mybir as mybir
    from concourse import bass2jax

    nc = _get_program()
    bass2jax.install_neuronx_cc_hook()
    assert nc.dbg_addr is None
    partition_name = (nc.partition_id_tensor.name
                      if nc.partition_id_tensor else None)

    in_names, out_names, out_shapes, out_dtypes = [], [], [], []
    for alloc in nc.m.functions[0].allocations:
        if not isinstance(alloc, mybir.MemoryLocationSet):
            continue
        name = alloc.memorylocations[0].name
        if alloc.kind == "ExternalInput":
            if name != partition_name:
                in_names.append(name)
        elif alloc.kind == "ExternalOutput":
            out_names.append(name)
            out_shapes.append(tuple(alloc.tensor_shape))
            out_dtypes.append(mybir.dt.np(alloc.dtype))
    out_avals = [jax.core.ShapedArray(s, d)
                 for s, d in zip(out_shapes, out_dtypes)]
    n_params = len(in_names)
    all_names = list(in_names + out_names)
    if partition_name is not None:
        all_names.append(partition_name)
    all_names = tuple(all_names)

    def _body(*args):
        operands = list(args)
        if partition_name is not None:
            operands.append(bass2jax.partition_id_tensor())
        outs = bass2jax._bass_exec_p.bind(
            *operands,
            out_avals=tuple(out_avals),
            in_names=all_names,
            out_names=tuple(out_names),
            lowering_input_output_aliases=(),
            sim_require_finite=True,
            sim_require_nnan=True,
            nc=nc,
        )
        return tuple(outs)

    devices = jax.devices()[:N_CORES]
    assert len(devices) == N_CORES
    mesh = Mesh(np.asarray(devices), ("core",))
    donate = tuple(range(n_params, n_params + len(out_names)))
    sharded = jax.jit(
        shard_map(
            _body, mesh=mesh,
            in_specs=(PartitionSpec("core"),) * (n_params + len(out_names)),
            out_specs=(PartitionSpec("core"),) * len(out_names),
            check_rep=False),
        donate_argnums=donate, keep_unused=True)

    def run(in_maps):
        per_core = [[np.asarray(m[name]) for name in in_names]
                    for m in in_maps]
        concat_in = [
            np.concatenate([per_core[c][i] for c in range(N_CORES)], axis=0)
            for i in range(n_params)
        ]
        zero_outs = [np.zeros((N_CORES * s[0],) + s[1:], d)
                     for s, d in zip(out_shapes, out_dtypes)]
        outs = sharded(*concat_in, *zero_outs)
        results = []
        for c in range(N_CORES):
            m = {}
            for i, name in enumerate(out_names):
                s0 = out_shapes[i][0]
                m[name] = np.asarray(outs[i][c * s0:(c + 1) * s0])
            results.append(m)
        return results

    _RUNNER = run
    return run


def kernel(x, weight_mu, weight_log_var, bias_mu, bias_log_var,
           weight_eps, bias_eps):
    in_maps = make_in_maps(x, weight_mu, weight_log_var, bias_mu,
                           bias_log_var, weight_eps, bias_eps)
    results = _get_runner()(in_maps)
    return gather_output(results)
